# revision 23
# baseline (speedup 1.0000x reference)
"""Multi-head causal self-attention (B=4, S=2048, D=512, H=8) on 8 Trainium2
NeuronCores.

Sharding: core c handles batch b = c//2 and a 4-head group g = c%2
(heads 4g..4g+3, i.e. output-feature slice [256g, 256g+256)).  Each core's
output is a disjoint slice of the full output, so no collectives are needed.

Device kernel layout choices (per core):
  - inputs are passed transposed+bf16 (xT = x.T : [din, S]) so the
    projection matmuls can contract din on the partition dim.
  - Q,K are produced transposed ("QT/KT" = [dout, S]); attention scores are
    computed transposed: ST[k, q] = sum_d KT[d,k] * QT[d,q], which makes the
    softmax denominator and PV matmul contract over k on partitions.
  - softmax skips the max-subtraction: logits = s/8 with |s/8| <~ 6 for this
    problem's N(0,1)-ish inputs, safely inside exp's fp32 range.  exp runs on
    the scalar engine straight out of PSUM.
  - V is augmented with a ones-column, so the PV matmul accumulates both
    out^T[dv, q] and the softmax denominator (row 64) in one pass.
  - normalization (divide by denom) + final transpose happen on the host
    during the gather step.
"""

import numpy as np
import ml_dtypes

from concourse import bacc, mybir
from concourse.tile import TileContext
from concourse.bass_utils import run_bass_kernel_spmd

BF16 = mybir.dt.bfloat16
F32 = mybir.dt.float32
AF = mybir.ActivationFunctionType
BFNP = ml_dtypes.bfloat16

B, S, D = 4, 2048, 512
H, HD = 8, 64
HPC = 4                   # heads per core
DSL = HPC * HD            # 256-wide output-feature slice per core
N_CORES = 8
SCALE = float(HD) ** 0.5  # 8.0
QH_W = 1024               # q processed in two halves of 1024


# timing instrumentation only: emit the compute body N times (identical
# output; wall-clock delta between variants isolates device compute time)
PASSES = 1


def build_nc():
    nc = bacc.Bacc("TRN2", target_bir_lowering=False)

    qT = nc.declare_dram_parameter("qT", [D, S], BF16, isOutput=False)
    kTd = nc.declare_dram_parameter("kTd", [D, S], BF16, isOutput=False)
    vT = nc.declare_dram_parameter("vT", [D, S], BF16, isOutput=False)
    wqT = nc.declare_dram_parameter("wqT", [D, DSL], BF16, isOutput=False)
    wkT = nc.declare_dram_parameter("wkT", [D, DSL], BF16, isOutput=False)
    wvT = nc.declare_dram_parameter("wvT", [D, DSL], BF16, isOutput=False)
    # packed small tensors: [0:2]=bq, [2:4]=bk, [4:260]=bvb, [260:324]=mask(bf16 bits)
    smallp = nc.declare_dram_parameter("smallp", [128, 324], F32, isOutput=False)
    # rows [65h, 65h+64) = unnormalized out^T for head h; row 65h+64 = denom
    out_t = nc.declare_dram_parameter(
        "out_t", [HPC * (HD + 1), S], F32, isOutput=True
    )

    with TileContext(nc) as tc:
        with tc.tile_pool(name="const", bufs=1) as cpool:
            qT_sb = cpool.tile([128, 4, S], BF16, tag="qT_sb")
            kT_sb = cpool.tile([128, 4, S], BF16, tag="kT_sb")
            vT_sb = cpool.tile([128, 4, S], BF16, tag="vT_sb")
            wq_sb = cpool.tile([128, 4, DSL], BF16, tag="wq_sb")
            wk_sb = cpool.tile([128, 4, DSL], BF16, tag="wk_sb")
            wv_sb = cpool.tile([128, 4, DSL], BF16, tag="wv_sb")
            small_sb = cpool.tile([128, 324], F32, tag="small_sb")
            bq_sb = small_sb[:, 0:2]
            bk_sb = small_sb[:, 2:4]
            bvb_sb = small_sb[:, 4:260]
            mask_sb = small_sb[:, 260:324].bitcast(BF16)
            # projected tensors: chunk dim = head pair (dout 128-chunk)
            QT_sb = cpool.tile([128, 2, S], BF16, tag="QT_sb")
            KT_sb = cpool.tile([128, 2, S], BF16, tag="KT_sb")
            # V with ones column: [k-part, head, k-tile, dv+1]
            vaug_sb = cpool.tile([128, HPC, 16, HD + 1], BF16, tag="vaug_sb")

            nc.vector.memset(vaug_sb[:], 1.0)

            def load_w(w_sb, wsrc, eng):
                eng.dma_start(w_sb[:], wsrc[:].rearrange("(c p) m -> p c m", p=128))

            def load_x(dstt, srcd, sq, eng):
                s0 = 512 * sq
                eng.dma_start(
                    dstt[:, :, s0 : s0 + 512],
                    srcd[:, s0 : s0 + 512].rearrange("(c p) s -> p c s", p=128),
                )

            # loads ordered by when the first attention tiles need them
            _Q, _K, _V = (qT_sb, qT), (kT_sb, kTd), (vT_sb, vT)
            load_w(wv_sb, wvT, nc.sync)
            load_x(*_V, 0, nc.sync)
            load_w(wk_sb, wkT, nc.sync)
            load_x(*_K, 0, nc.sync)
            nc.sync.dma_start(small_sb[:], smallp[:])
            load_w(wq_sb, wqT, nc.sync)
            load_x(*_Q, 0, nc.sync)
            load_x(*_Q, 1, nc.sync)
            for xt, sq in ((_K, 1), (_V, 1), (_Q, 2), (_Q, 3),
                           (_K, 2), (_K, 3), (_V, 2), (_V, 3)):
                load_x(*xt, sq, nc.sync)

            # ---- projections + attention, interleaved ----
            # PSUM budget: ppool 2x1 + spool 2x2 + apool 1x2 = 8 banks
            with (
                tc.tile_pool(name="ppsum", bufs=2, space="PSUM") as ppool,
                tc.tile_pool(name="spsum", bufs=2, space="PSUM") as spool,
                tc.tile_pool(name="apsum", bufs=1, space="PSUM") as apool,
                tc.tile_pool(name="epool", bufs=5) as epool,
                tc.tile_pool(name="opool", bufs=2) as opool,
            ):

                def proj_v_st(st):
                    ps = ppool.tile([128, 512], F32, tag="pproj", name="psv")
                    for dc in range(4):
                        nc.tensor.matmul(
                            ps[:, 0:DSL],
                            vT_sb[:, dc, 128 * st : 128 * st + 128],
                            wv_sb[:, dc, :],
                            start=(dc == 0),
                            stop=(dc == 3),
                        )
                    for hh in range(HPC):
                        nc.vector.tensor_add(
                            vaug_sb[:, hh, st, 0:HD],
                            ps[:, HD * hh : HD * hh + HD],
                            bvb_sb[:, HD * hh : HD * hh + HD],
                        )

                QSRC = (wq_sb, bq_sb, qT_sb, QT_sb)
                KSRC = (wk_sb, bk_sb, kT_sb, KT_sb)

                def proj_qk_tile(mc, sc, src):
                    w_sb, b_sb, x_sb, dst = src
                    ps = ppool.tile([128, 512], F32, tag="pproj", name="psqk")
                    for dc in range(4):
                        nc.tensor.matmul(
                            ps[:],
                            w_sb[:, dc, 128 * mc : 128 * mc + 128],
                            x_sb[:, dc, 512 * sc : 512 * sc + 512],
                            start=(dc == 0),
                            stop=(dc == 3),
                        )
                    nc.vector.tensor_scalar_add(
                        dst[:, mc, 512 * sc : 512 * sc + 512],
                        ps[:],
                        b_sb[:, mc : mc + 1],
                    )

                def proj_qk(mc):
                    # order tiles by when head (2*mc)'s attention needs them
                    for sc, src in (
                        (0, KSRC), (0, QSRC), (1, QSRC), (1, KSRC),
                        (2, QSRC), (2, KSRC), (3, QSRC), (3, KSRC),
                    ):
                        proj_qk_tile(mc, sc, src)

                def attn_head(h, background=None, stride=4):
                    background = background if background is not None else []
                    eligible = 0
                    mc, prow = h // 2, 64 * (h % 2)
                    for qh in range(2):
                        Q0 = QH_W * qh
                        kmax = 8 if qh == 0 else 16
                        acc = apool.tile([HD + 1, QH_W], F32, tag="acc", name="acc")

                        def scores(kt):
                            K0 = 128 * kt
                            qlo = max(Q0, K0)
                            W = Q0 + QH_W - qlo
                            sl = spool.tile([128, QH_W], F32, tag="sl", name="sl")
                            for c0 in range(0, W, 512):
                                cw = min(512, W - c0)
                                nc.tensor.matmul(
                                    sl[:, c0 : c0 + cw],
                                    KT_sb[prow : prow + 64, mc, K0 : K0 + 128],
                                    QT_sb[
                                        prow : prow + 64, mc, qlo + c0 : qlo + c0 + cw
                                    ],
                                    start=True,
                                    stop=True,
                                )
                            return sl

                        # software pipeline: scores run one kt ahead of PV
                        sl = scores(0)
                        for kt in range(kmax):
                            K0 = 128 * kt
                            qlo = max(Q0, K0)
                            W = Q0 + QH_W - qlo
                            off = qlo - Q0
                            et = epool.tile([128, QH_W], BF16, tag="et", name="et")
                            nc.scalar.activation(
                                et[:, 0:W], sl[:, 0:W], AF.Exp, scale=1.0 / SCALE
                            )
                            if kt + 1 < kmax:
                                sl = scores(kt + 1)
                            if h == 0 and (qh == 0 or kt >= 8) and kt >= 4:
                                proj_v_st(kt)  # st == kt; fills vaug for PV below
                            elif background:
                                if eligible % stride == 0:
                                    background.pop(0)()  # deferred projection tile
                                eligible += 1
                            if K0 >= Q0:
                                nc.vector.tensor_mul(
                                    et[:, 0:128], et[:, 0:128], mask_sb[:]
                                )
                            b0 = off
                            while b0 < QH_W:
                                b1 = min(QH_W, (b0 // 512 + 1) * 512)
                                nc.tensor.matmul(
                                    acc[:, b0:b1],
                                    vaug_sb[:, h, kt, :],
                                    et[:, b0 - off : b1 - off],
                                    start=(kt == 0),
                                    stop=(kt == kmax - 1),
                                    skip_group_check=True,
                                )
                                b0 = b1
                        ot = opool.tile([HD + 1, QH_W], F32, tag="ot", name="ot")
                        nc.vector.tensor_copy(ot[:], acc[:])
                        nc.sync.dma_start(
                            out_t[(HD + 1) * h : (HD + 1) * h + HD + 1, Q0 : Q0 + QH_W],
                            ot[:],
                        )

                for _pass in range(PASSES):
                    for st in range(4):
                        proj_v_st(st)  # runs in the initial DMA-wait window
                    proj_qk(0)
                    qk1_tiles = [
                        (lambda sc=sc, s=s: proj_qk_tile(1, sc, s))
                        for sc, s in (
                            (0, KSRC), (0, QSRC), (1, QSRC), (1, KSRC),
                            (2, QSRC), (2, KSRC), (3, QSRC), (3, KSRC),
                        )
                    ]
                    attn_head(0, background=qk1_tiles, stride=3)
                    attn_head(1, background=qk1_tiles, stride=3)
                    for work in qk1_tiles:
                        work()  # any leftovers
                    attn_head(2)
                    attn_head(3)

    nc.finalize()
    return nc


_NC_CACHE = {}


def _get_nc():
    if "nc" not in _NC_CACHE:
        _NC_CACHE["nc"] = build_nc()
    return _NC_CACHE["nc"]


def make_in_maps(query, key, value, Wq, bq, Wk, bk, Wv, bv):
    query, key, value = (np.asarray(x, np.float32) for x in (query, key, value))
    Wq, Wk, Wv = (np.asarray(x, np.float32) for x in (Wq, Wk, Wv))
    bq, bk, bv = (np.asarray(x, np.float32) for x in (bq, bk, bv))
    mask = np.triu(np.ones((128, 128), np.float32)).astype(BFNP)

    def pack_small(bqs, bks, bvs, m):
        out = np.empty((128, 324), np.float32)
        out[:, 0:2] = bqs.reshape(2, 128).T
        out[:, 2:4] = bks.reshape(2, 128).T
        out[:, 4:260] = np.tile(bvs[None, :], (128, 1))
        out[:, 260:324] = np.ascontiguousarray(m).view(np.float32)
        return out

    in_maps = []
    for c in range(N_CORES):
        b, g = c // 2, c % 2
        sl = slice(DSL * g, DSL * g + DSL)
        in_maps.append(
            {
                "qT": np.ascontiguousarray(query[b].astype(BFNP).T),
                "kTd": np.ascontiguousarray(key[b].astype(BFNP).T),
                "vT": np.ascontiguousarray(value[b].astype(BFNP).T),
                "wqT": np.ascontiguousarray(Wq[sl].astype(BFNP).T),
                "wkT": np.ascontiguousarray(Wk[sl].astype(BFNP).T),
                "wvT": np.ascontiguousarray(Wv[sl].astype(BFNP).T),
                "smallp": pack_small(bq[sl], bk[sl], bv[sl], mask),
            }
        )
    return in_maps


def assemble_output(results):
    out = np.empty((B, S, D), np.float32)
    for c in range(N_CORES):
        b, g = c // 2, c % 2
        ot = results[c]["out_t"]  # [260, 2048]
        for hl in range(HPC):
            blk = ot[(HD + 1) * hl : (HD + 1) * hl + HD]  # [64, S]
            den = ot[(HD + 1) * hl + HD]  # [S]
            h = HPC * g + hl
            out[b, :, HD * h : HD * h + HD] = (blk / den).T
    return out


def run(trace=False, **inputs):
    nc = _get_nc()
    in_maps = make_in_maps(**inputs)
    res = run_bass_kernel_spmd(nc, in_maps, list(range(N_CORES)), trace=trace)
    return assemble_output(res.results), res


def kernel(**inputs) -> np.ndarray:
    out, _ = run(trace=False, **inputs)
    return out


# revision 30
# speedup vs baseline: 1.1010x; 1.1010x over previous
"""Multi-head causal self-attention (B=4, S=2048, D=512, H=8) on 8 Trainium2
NeuronCores.

Sharding: core c handles batch b = c//2 and a 4-head group g = c%2
(heads 4g..4g+3, i.e. output-feature slice [256g, 256g+256)).  Each core's
output is a disjoint slice of the full output, so no collectives are needed.

Device kernel layout choices (per core):
  - inputs are passed transposed+bf16 (xT = x.T : [din, S]) so the
    projection matmuls can contract din on the partition dim.
  - Q,K are produced transposed ("QT/KT" = [dout, S]); attention scores are
    computed transposed: ST[k, q] = sum_d KT[d,k] * QT[d,q], which makes the
    softmax denominator and PV matmul contract over k on partitions.
  - softmax skips the max-subtraction: logits = s/8 with |s/8| <~ 6 for this
    problem's N(0,1)-ish inputs, safely inside exp's fp32 range.  exp runs on
    the scalar engine straight out of PSUM.
  - V is augmented with a ones-column, so the PV matmul accumulates both
    out^T[dv, q] and the softmax denominator (row 64) in one pass.
  - normalization (divide by denom) + final transpose happen on the host
    during the gather step.
"""

import numpy as np
import ml_dtypes

from concourse import bacc, mybir
from concourse.tile import TileContext
from concourse.bass_utils import run_bass_kernel_spmd

BF16 = mybir.dt.bfloat16
F32 = mybir.dt.float32
AF = mybir.ActivationFunctionType
BFNP = ml_dtypes.bfloat16

B, S, D = 4, 2048, 512
H, HD = 8, 64
HPC = 4                   # heads per core
DSL = HPC * HD            # 256-wide output-feature slice per core
N_CORES = 8
SCALE = float(HD) ** 0.5  # 8.0
QH_W = 1024               # q processed in two halves of 1024


# timing instrumentation only: emit the compute body N times (identical
# output; wall-clock delta between variants isolates device compute time)
PASSES = 1


def build_nc():
    nc = bacc.Bacc("TRN2", target_bir_lowering=False)

    qT = nc.declare_dram_parameter("qT", [D, S], BF16, isOutput=False)
    kTd = nc.declare_dram_parameter("kTd", [D, S], BF16, isOutput=False)
    vT = nc.declare_dram_parameter("vT", [D, S], BF16, isOutput=False)
    wqT = nc.declare_dram_parameter("wqT", [D, DSL], BF16, isOutput=False)
    wkT = nc.declare_dram_parameter("wkT", [D, DSL], BF16, isOutput=False)
    wvT = nc.declare_dram_parameter("wvT", [D, DSL], BF16, isOutput=False)
    # packed small tensors: [0:2]=bq, [2:4]=bk, [4:260]=bvb, [260:324]=mask(bf16 bits)
    smallp = nc.declare_dram_parameter("smallp", [128, 324], F32, isOutput=False)
    # rows [65h, 65h+64) = unnormalized out^T for head h; row 65h+64 = denom
    out_t = nc.declare_dram_parameter(
        "out_t", [HPC * (HD + 1), S], F32, isOutput=True
    )

    with TileContext(nc) as tc:
        with tc.tile_pool(name="const", bufs=1) as cpool:
            qT_sb = cpool.tile([128, 4, S], BF16, tag="qT_sb")
            kT_sb = cpool.tile([128, 4, S], BF16, tag="kT_sb")
            vT_sb = cpool.tile([128, 4, S], BF16, tag="vT_sb")
            wq_sb = cpool.tile([128, 4, DSL], BF16, tag="wq_sb")
            wk_sb = cpool.tile([128, 4, DSL], BF16, tag="wk_sb")
            wv_sb = cpool.tile([128, 4, DSL], BF16, tag="wv_sb")
            small_sb = cpool.tile([128, 324], F32, tag="small_sb")
            bq_sb = small_sb[:, 0:2]
            bk_sb = small_sb[:, 2:4]
            bvb_sb = small_sb[:, 4:260]
            mask_sb = small_sb[:, 260:324].bitcast(BF16)
            # projected tensors: chunk dim = head pair (dout 128-chunk)
            QT_sb = cpool.tile([128, 2, S], BF16, tag="QT_sb")
            KT_sb = cpool.tile([128, 2, S], BF16, tag="KT_sb")
            # V with ones column: [k-part, head, k-tile, dv+1]
            vaug_sb = cpool.tile([128, HPC, 16, HD + 1], BF16, tag="vaug_sb")

            # only the ones-column needs init; cols 0..63 are written by proj_v
            nc.vector.memset(vaug_sb[:, :, :, HD : HD + 1], 1.0)

            def load_w(w_sb, wsrc, eng):
                eng.dma_start(w_sb[:], wsrc[:].rearrange("(c p) m -> p c m", p=128))

            def load_x(dstt, srcd, sq, eng):
                s0 = 512 * sq
                eng.dma_start(
                    dstt[:, :, s0 : s0 + 512],
                    srcd[:, s0 : s0 + 512].rearrange("(c p) s -> p c s", p=128),
                )

            # loads ordered by when the first attention tiles need them
            _Q, _K, _V = (qT_sb, qT), (kT_sb, kTd), (vT_sb, vT)
            load_w(wv_sb, wvT, nc.sync)
            load_x(*_V, 0, nc.sync)
            load_w(wk_sb, wkT, nc.sync)
            load_x(*_K, 0, nc.sync)
            nc.sync.dma_start(small_sb[:], smallp[:])
            load_w(wq_sb, wqT, nc.sync)
            load_x(*_Q, 0, nc.sync)
            load_x(*_Q, 1, nc.sync)
            for xt, sq in ((_K, 1), (_V, 1), (_Q, 2), (_Q, 3),
                           (_K, 2), (_K, 3), (_V, 2), (_V, 3)):
                load_x(*xt, sq, nc.sync)

            # ---- projections + attention, interleaved ----
            # PSUM budget: ppool 2x1 + spool 2x2 + apool 1x2 = 8 banks
            with (
                tc.tile_pool(name="ppsum", bufs=2, space="PSUM") as ppool,
                tc.tile_pool(name="spsum", bufs=2, space="PSUM") as spool,
                tc.tile_pool(name="apsum", bufs=1, space="PSUM") as apool,
                tc.tile_pool(name="epool", bufs=7) as epool,
                tc.tile_pool(name="opool", bufs=3) as opool,
            ):

                def proj_v_st(st):
                    ps = ppool.tile([128, 512], F32, tag="pproj", name="psv")
                    for dc in range(4):
                        nc.tensor.matmul(
                            ps[:, 0:DSL],
                            vT_sb[:, dc, 128 * st : 128 * st + 128],
                            wv_sb[:, dc, :],
                            start=(dc == 0),
                            stop=(dc == 3),
                        )
                    for hh in range(HPC):
                        nc.vector.tensor_add(
                            vaug_sb[:, hh, st, 0:HD],
                            ps[:, HD * hh : HD * hh + HD],
                            bvb_sb[:, HD * hh : HD * hh + HD],
                        )

                QSRC = (wq_sb, bq_sb, qT_sb, QT_sb)
                KSRC = (wk_sb, bk_sb, kT_sb, KT_sb)

                def proj_qk_tile(mc, sc, src):
                    w_sb, b_sb, x_sb, dst = src
                    ps = ppool.tile([128, 512], F32, tag="pproj", name="psqk")
                    for dc in range(4):
                        nc.tensor.matmul(
                            ps[:],
                            w_sb[:, dc, 128 * mc : 128 * mc + 128],
                            x_sb[:, dc, 512 * sc : 512 * sc + 512],
                            start=(dc == 0),
                            stop=(dc == 3),
                        )
                    nc.vector.tensor_scalar_add(
                        dst[:, mc, 512 * sc : 512 * sc + 512],
                        ps[:],
                        b_sb[:, mc : mc + 1],
                    )

                def attn_head(h, sched=None):
                    sched = sched or {}
                    mc, prow = h // 2, 64 * (h % 2)
                    for qh in range(2):
                        Q0 = QH_W * qh
                        kmax = 8 if qh == 0 else 16
                        # late (narrow) kt tiles share one slab at 512-aligned
                        # offsets -> one exp instruction per group
                        if qh == 0:
                            groups = [(0,), (1,), (2,), (3,), (4, 5), (6, 7)]
                        else:
                            groups = [(k,) for k in range(12)] + [(12, 13), (14, 15)]
                        acc = apool.tile([HD + 1, QH_W], F32, tag="acc", name="acc")

                        def geom(kt):
                            K0 = 128 * kt
                            qlo = max(Q0, K0)
                            return K0, qlo, Q0 + QH_W - qlo

                        def grp_offsets(grp):
                            # pack members tightly; a scores region must not
                            # cross a 512-element PSUM bank boundary
                            pos, offs = 0, []
                            for kt in grp:
                                W = geom(kt)[2]
                                if pos % 512 + min(W, 512) > 512:
                                    pos = (pos + 511) // 512 * 512
                                offs.append(pos)
                                pos += W
                            return offs, pos

                        def scores_grp(gi):
                            sl = spool.tile([128, QH_W], F32, tag="sl", name="sl")
                            offs = grp_offsets(groups[gi])[0]
                            for j, kt in enumerate(groups[gi]):
                                K0, qlo, W = geom(kt)
                                base = offs[j]
                                for c0 in range(0, W, 512):
                                    cw = min(512, W - c0)
                                    nc.tensor.matmul(
                                        sl[:, base + c0 : base + c0 + cw],
                                        KT_sb[prow : prow + 64, mc, K0 : K0 + 128],
                                        QT_sb[
                                            prow : prow + 64, mc,
                                            qlo + c0 : qlo + c0 + cw,
                                        ],
                                        start=True,
                                        stop=True,
                                    )
                            return sl

                        # software pipeline: scores run one group ahead of PV
                        sl = scores_grp(0)
                        for gi, grp in enumerate(groups):
                            goffs, We = grp_offsets(grp)
                            et = epool.tile([128, QH_W], BF16, tag="et", name="et")
                            nc.scalar.activation(
                                et[:, 0:We], sl[:, 0:We], AF.Exp, scale=1.0 / SCALE
                            )
                            if gi + 1 < len(groups):
                                sl = scores_grp(gi + 1)
                            for kt in grp:
                                if h == 0 and (qh == 0 or kt >= 8) and kt >= 4:
                                    proj_v_st(kt)  # st == kt; fills vaug for PV
                                for work in sched.get((qh, kt), ()):
                                    work()  # deferred projection tile
                            for j, kt in enumerate(grp):
                                K0, qlo, W = geom(kt)
                                off = qlo - Q0
                                base = goffs[j]
                                if K0 >= Q0:
                                    nc.vector.tensor_mul(
                                        et[:, base : base + 128],
                                        et[:, base : base + 128],
                                        mask_sb[:],
                                    )
                                b0 = off
                                while b0 < QH_W:
                                    b1 = min(QH_W, (b0 // 512 + 1) * 512)
                                    nc.tensor.matmul(
                                        acc[:, b0:b1],
                                        vaug_sb[:, h, kt, :],
                                        et[:, base + b0 - off : base + b1 - off],
                                        start=(kt == 0),
                                        stop=(kt == kmax - 1),
                                        skip_group_check=True,
                                    )
                                    b0 = b1
                        ot = opool.tile([HD + 1, QH_W], F32, tag="ot", name="ot")
                        nc.vector.tensor_copy(ot[:], acc[:])
                        nc.sync.dma_start(
                            out_t[(HD + 1) * h : (HD + 1) * h + HD + 1, Q0 : Q0 + QH_W],
                            ot[:],
                        )

                def qk_tile(mc, sc, s):
                    return lambda: proj_qk_tile(mc, sc, s)

                for _pass in range(PASSES):
                    # prologue: only the tiles the first scores/PV need, V-proj
                    # interleaved to fill DMA-wait bubbles
                    proj_v_st(0)
                    proj_v_st(1)
                    proj_qk_tile(0, 0, KSRC)
                    proj_qk_tile(0, 0, QSRC)
                    proj_qk_tile(0, 1, QSRC)
                    proj_v_st(2)
                    proj_v_st(3)
                    q1 = [qk_tile(1, sc, s) for sc, s in (
                        (0, KSRC), (0, QSRC), (1, QSRC), (1, KSRC),
                        (2, QSRC), (2, KSRC), (3, QSRC), (3, KSRC))]
                    # deferred tiles, placed just before their deadlines in
                    # windows where ACT (exp) is the busier engine
                    attn_head(0, sched={
                        (0, 0): [qk_tile(0, 1, KSRC)],
                        (0, 1): [qk_tile(0, 2, QSRC)],
                        (0, 2): [qk_tile(0, 3, QSRC)],
                        (1, 0): [qk_tile(0, 2, KSRC)],
                        (1, 1): [qk_tile(0, 3, KSRC)],
                        (1, 2): [q1[0]], (1, 3): [q1[1]], (1, 4): [q1[2]],
                        (1, 5): [q1[3]], (1, 6): [q1[4]], (1, 7): [q1[5]],
                    })
                    attn_head(1, sched={(0, 0): [q1[6]], (0, 1): [q1[7]]})
                    attn_head(2)
                    attn_head(3)

    nc.finalize()
    return nc


_NC_CACHE = {}


def _get_nc():
    if "nc" not in _NC_CACHE:
        _NC_CACHE["nc"] = build_nc()
    return _NC_CACHE["nc"]


def make_in_maps(query, key, value, Wq, bq, Wk, bk, Wv, bv):
    query, key, value = (np.asarray(x, np.float32) for x in (query, key, value))
    Wq, Wk, Wv = (np.asarray(x, np.float32) for x in (Wq, Wk, Wv))
    bq, bk, bv = (np.asarray(x, np.float32) for x in (bq, bk, bv))
    mask = np.triu(np.ones((128, 128), np.float32)).astype(BFNP)

    def pack_small(bqs, bks, bvs, m):
        out = np.empty((128, 324), np.float32)
        out[:, 0:2] = bqs.reshape(2, 128).T
        out[:, 2:4] = bks.reshape(2, 128).T
        out[:, 4:260] = np.tile(bvs[None, :], (128, 1))
        out[:, 260:324] = np.ascontiguousarray(m).view(np.float32)
        return out

    in_maps = []
    for c in range(N_CORES):
        b, g = c // 2, c % 2
        sl = slice(DSL * g, DSL * g + DSL)
        in_maps.append(
            {
                "qT": np.ascontiguousarray(query[b].astype(BFNP).T),
                "kTd": np.ascontiguousarray(key[b].astype(BFNP).T),
                "vT": np.ascontiguousarray(value[b].astype(BFNP).T),
                "wqT": np.ascontiguousarray(Wq[sl].astype(BFNP).T),
                "wkT": np.ascontiguousarray(Wk[sl].astype(BFNP).T),
                "wvT": np.ascontiguousarray(Wv[sl].astype(BFNP).T),
                "smallp": pack_small(bq[sl], bk[sl], bv[sl], mask),
            }
        )
    return in_maps


def assemble_output(results):
    out = np.empty((B, S, D), np.float32)
    for c in range(N_CORES):
        b, g = c // 2, c % 2
        ot = results[c]["out_t"]  # [260, 2048]
        for hl in range(HPC):
            blk = ot[(HD + 1) * hl : (HD + 1) * hl + HD]  # [64, S]
            den = ot[(HD + 1) * hl + HD]  # [S]
            h = HPC * g + hl
            out[b, :, HD * h : HD * h + HD] = (blk / den).T
    return out


def run(trace=False, **inputs):
    nc = _get_nc()
    in_maps = make_in_maps(**inputs)
    res = run_bass_kernel_spmd(nc, in_maps, list(range(N_CORES)), trace=trace)
    return assemble_output(res.results), res


def kernel(**inputs) -> np.ndarray:
    out, _ = run(trace=False, **inputs)
    return out


# revision 36
# speedup vs baseline: 1.1103x; 1.0084x over previous
"""Multi-head causal self-attention (B=4, S=2048, D=512, H=8) on 8 Trainium2
NeuronCores.

Sharding: core c handles batch b = c//2 and a 4-head group g = c%2
(heads 4g..4g+3, i.e. output-feature slice [256g, 256g+256)).  Each core's
output is a disjoint slice of the full output, so no collectives are needed.

Device kernel layout choices (per core):
  - inputs are passed transposed+bf16 (xT = x.T : [din, S]) so the
    projection matmuls can contract din on the partition dim.
  - Q,K are produced transposed ("QT/KT" = [dout, S]); attention scores are
    computed transposed: ST[k, q] = sum_d KT[d,k] * QT[d,q], which makes the
    softmax denominator and PV matmul contract over k on partitions.
  - softmax skips the max-subtraction: logits = s/8 with |s/8| <~ 6 for this
    problem's N(0,1)-ish inputs, safely inside exp's fp32 range.  exp runs on
    the scalar engine straight out of PSUM.
  - V is augmented with a ones-column, so the PV matmul accumulates both
    out^T[dv, q] and the softmax denominator (row 64) in one pass.
  - normalization (divide by denom) + final transpose happen on the host
    during the gather step.
"""

import numpy as np
import ml_dtypes

from concourse import bacc, mybir
from concourse.tile import TileContext
from concourse.bass_utils import run_bass_kernel_spmd

BF16 = mybir.dt.bfloat16
F32 = mybir.dt.float32
AF = mybir.ActivationFunctionType
BFNP = ml_dtypes.bfloat16

B, S, D = 4, 2048, 512
H, HD = 8, 64
HPC = 4                   # heads per core
DSL = HPC * HD            # 256-wide output-feature slice per core
N_CORES = 8
SCALE = float(HD) ** 0.5  # 8.0
QH_W = 1024               # q processed in two halves of 1024


# timing instrumentation only: emit the compute body N times (identical
# output; wall-clock delta between variants isolates device compute time)
PASSES = 1


def build_nc():
    nc = bacc.Bacc("TRN2", target_bir_lowering=False)

    qT = nc.declare_dram_parameter("qT", [D, S], BF16, isOutput=False)
    kTd = nc.declare_dram_parameter("kTd", [D, S], BF16, isOutput=False)
    vT = nc.declare_dram_parameter("vT", [D, S], BF16, isOutput=False)
    wqT = nc.declare_dram_parameter("wqT", [D, DSL], BF16, isOutput=False)
    wkT = nc.declare_dram_parameter("wkT", [D, DSL], BF16, isOutput=False)
    wvT = nc.declare_dram_parameter("wvT", [D, DSL], BF16, isOutput=False)
    # packed small tensors: [0:2]=bq, [2:4]=bk, [4:260]=bvb, [260:324]=mask(bf16 bits)
    smallp = nc.declare_dram_parameter("smallp", [128, 324], F32, isOutput=False)
    # rows [65h, 65h+64) = unnormalized out^T for head h; row 65h+64 = denom
    out_t = nc.declare_dram_parameter(
        "out_t", [HPC * (HD + 1), S], F32, isOutput=True
    )

    with TileContext(nc) as tc:
        with tc.tile_pool(name="const", bufs=1) as cpool:
            qT_sb = cpool.tile([128, 4, S], BF16, tag="qT_sb")
            kT_sb = cpool.tile([128, 4, S], BF16, tag="kT_sb")
            vT_sb = cpool.tile([128, 4, S], BF16, tag="vT_sb")
            wq_sb = cpool.tile([128, 4, DSL], BF16, tag="wq_sb")
            wk_sb = cpool.tile([128, 4, DSL], BF16, tag="wk_sb")
            wv_sb = cpool.tile([128, 4, DSL], BF16, tag="wv_sb")
            small_sb = cpool.tile([128, 324], F32, tag="small_sb")
            bq_sb = small_sb[:, 0:2]
            bk_sb = small_sb[:, 2:4]
            bvb_sb = small_sb[:, 4:260]
            mask_sb = small_sb[:, 260:324].bitcast(BF16)
            # projected tensors: chunk dim = head pair (dout 128-chunk)
            QT_sb = cpool.tile([128, 2, S], BF16, tag="QT_sb")
            KT_sb = cpool.tile([128, 2, S], BF16, tag="KT_sb")
            # V with ones column: [k-part, head, k-tile, dv+1]
            vaug_sb = cpool.tile([128, HPC, 16, HD + 1], BF16, tag="vaug_sb")

            # only the ones-column needs init; cols 0..63 are written by proj_v
            nc.vector.memset(vaug_sb[:, :, :, HD : HD + 1], 1.0)

            def load_w(w_sb, wsrc, eng):
                eng.dma_start(w_sb[:], wsrc[:].rearrange("(c p) m -> p c m", p=128))

            def load_x(dstt, srcd, sq, eng):
                s0 = 512 * sq
                eng.dma_start(
                    dstt[:, :, s0 : s0 + 512],
                    srcd[:, s0 : s0 + 512].rearrange("(c p) s -> p c s", p=128),
                )

            # loads ordered by when the first attention tiles need them
            _Q, _K, _V = (qT_sb, qT), (kT_sb, kTd), (vT_sb, vT)
            load_w(wv_sb, wvT, nc.sync)
            load_x(*_V, 0, nc.sync)
            load_w(wk_sb, wkT, nc.sync)
            load_x(*_K, 0, nc.sync)
            nc.sync.dma_start(small_sb[:], smallp[:])
            load_w(wq_sb, wqT, nc.sync)
            load_x(*_Q, 0, nc.sync)
            load_x(*_Q, 1, nc.sync)
            for xt, sq in ((_K, 1), (_V, 1), (_Q, 2), (_Q, 3),
                           (_K, 2), (_K, 3), (_V, 2), (_V, 3)):
                load_x(*xt, sq, nc.sync)

            # ---- projections + attention, interleaved ----
            # PSUM budget: ppool 2x1 + spool 2x2 + apool 1x2 = 8 banks
            with (
                tc.tile_pool(name="ppsum", bufs=2, space="PSUM") as ppool,
                tc.tile_pool(name="spsum", bufs=2, space="PSUM") as spool,
                tc.tile_pool(name="apsum", bufs=1, space="PSUM") as apool,
                tc.tile_pool(name="epool", bufs=7) as epool,
                tc.tile_pool(name="opool", bufs=3) as opool,
            ):

                def proj_v_st(st):
                    ps = ppool.tile([128, 512], F32, tag="pproj", name="psv")
                    for dc in range(4):
                        nc.tensor.matmul(
                            ps[:, 0:DSL],
                            vT_sb[:, dc, 128 * st : 128 * st + 128],
                            wv_sb[:, dc, :],
                            start=(dc == 0),
                            stop=(dc == 3),
                        )
                    for hh in range(HPC):
                        nc.vector.tensor_add(
                            vaug_sb[:, hh, st, 0:HD],
                            ps[:, HD * hh : HD * hh + HD],
                            bvb_sb[:, HD * hh : HD * hh + HD],
                        )

                QSRC = (wq_sb, bq_sb, qT_sb, QT_sb)
                KSRC = (wk_sb, bk_sb, kT_sb, KT_sb)

                def proj_qk_tile(mc, sc, src):
                    w_sb, b_sb, x_sb, dst = src
                    ps = ppool.tile([128, 512], F32, tag="pproj", name="psqk")
                    for dc in range(4):
                        nc.tensor.matmul(
                            ps[:],
                            w_sb[:, dc, 128 * mc : 128 * mc + 128],
                            x_sb[:, dc, 512 * sc : 512 * sc + 512],
                            start=(dc == 0),
                            stop=(dc == 3),
                        )
                    nc.vector.tensor_scalar_add(
                        dst[:, mc, 512 * sc : 512 * sc + 512],
                        ps[:],
                        b_sb[:, mc : mc + 1],
                    )

                def attn_head(h, sched=None):
                    sched = sched or {}
                    mc, prow = h // 2, 64 * (h % 2)
                    GROUPS = {
                        0: [(0,), (1,), (2,), (3,), (4, 5), (6, 7)],
                        1: [(k,) for k in range(12)] + [(12, 13), (14, 15)],
                    }

                    def geom(qh, kt):
                        Q0 = QH_W * qh
                        K0 = 128 * kt
                        qlo = max(Q0, K0)
                        return K0, qlo, Q0 + QH_W - qlo

                    def grp_offsets(qh, grp):
                        # pack members tightly; a scores region must not
                        # cross a 512-element PSUM bank boundary
                        pos, offs = 0, []
                        for kt in grp:
                            W = geom(qh, kt)[2]
                            if pos % 512 + min(W, 512) > 512:
                                pos = (pos + 511) // 512 * 512
                            offs.append(pos)
                            pos += W
                        return offs, pos

                    def scores_grp(qh, gi):
                        sl = spool.tile([128, QH_W], F32, tag="sl", name="sl")
                        offs = grp_offsets(qh, GROUPS[qh][gi])[0]
                        for j, kt in enumerate(GROUPS[qh][gi]):
                            K0, qlo, W = geom(qh, kt)
                            base = offs[j]
                            for c0 in range(0, W, 512):
                                cw = min(512, W - c0)
                                nc.tensor.matmul(
                                    sl[:, base + c0 : base + c0 + cw],
                                    KT_sb[prow : prow + 64, mc, K0 : K0 + 128],
                                    QT_sb[
                                        prow : prow + 64, mc,
                                        qlo + c0 : qlo + c0 + cw,
                                    ],
                                    start=True,
                                    stop=True,
                                )
                        return sl

                    hoisted = None
                    for qh in range(2):
                        Q0 = QH_W * qh
                        kmax = 8 if qh == 0 else 16
                        groups = GROUPS[qh]
                        acc = apool.tile([HD + 1, QH_W], F32, tag="acc", name="acc")
                        # software pipeline: scores run one group ahead of PV
                        sl = hoisted if hoisted is not None else scores_grp(qh, 0)
                        hoisted = None
                        for gi, grp in enumerate(groups):
                            goffs, We = grp_offsets(qh, grp)
                            et = epool.tile([128, QH_W], BF16, tag="et", name="et")
                            nc.scalar.activation(
                                et[:, 0:We], sl[:, 0:We], AF.Exp, scale=1.0 / SCALE
                            )
                            if gi + 1 < len(groups):
                                sl = scores_grp(qh, gi + 1)
                            elif qh == 0:
                                # hoist next q-half's first scores ahead of
                                # this group's trailing PV matmuls
                                hoisted = scores_grp(1, 0)
                            for kt in grp:
                                if h == 0 and (qh == 0 or kt >= 8) and kt >= 4:
                                    proj_v_st(kt)  # st == kt; fills vaug for PV
                                for work in sched.get((qh, kt), ()):
                                    work()  # deferred projection tile
                            for j, kt in enumerate(grp):
                                K0, qlo, W = geom(qh, kt)
                                off = qlo - Q0
                                base = goffs[j]
                                if K0 >= Q0:
                                    nc.vector.tensor_mul(
                                        et[:, base : base + 128],
                                        et[:, base : base + 128],
                                        mask_sb[:],
                                    )
                                b0 = off
                                while b0 < QH_W:
                                    b1 = min(QH_W, (b0 // 512 + 1) * 512)
                                    nc.tensor.matmul(
                                        acc[:, b0:b1],
                                        vaug_sb[:, h, kt, :],
                                        et[:, base + b0 - off : base + b1 - off],
                                        start=(kt == 0),
                                        stop=(kt == kmax - 1),
                                        skip_group_check=True,
                                    )
                                    b0 = b1
                        ot = opool.tile([HD + 1, QH_W], F32, tag="ot", name="ot")
                        nc.vector.tensor_copy(ot[:], acc[:])
                        nc.sync.dma_start(
                            out_t[(HD + 1) * h : (HD + 1) * h + HD + 1, Q0 : Q0 + QH_W],
                            ot[:],
                        )

                def qk_tile(mc, sc, s):
                    return lambda: proj_qk_tile(mc, sc, s)

                for _pass in range(PASSES):
                    # prologue: only the tiles the first scores/PV need, V-proj
                    # interleaved to fill DMA-wait bubbles
                    proj_v_st(0)
                    proj_v_st(1)
                    proj_qk_tile(0, 0, KSRC)
                    proj_qk_tile(0, 0, QSRC)
                    proj_qk_tile(0, 1, QSRC)
                    proj_v_st(2)
                    proj_v_st(3)
                    q1 = [qk_tile(1, sc, s) for sc, s in (
                        (0, KSRC), (0, QSRC), (1, QSRC), (1, KSRC),
                        (2, QSRC), (2, KSRC), (3, QSRC), (3, KSRC))]
                    # deferred tiles, placed just before their deadlines in
                    # windows where ACT (exp) is the busier engine
                    attn_head(0, sched={
                        (0, 0): [qk_tile(0, 1, KSRC)],
                        (0, 1): [qk_tile(0, 2, QSRC)],
                        (0, 2): [qk_tile(0, 3, QSRC)],
                        (1, 0): [qk_tile(0, 2, KSRC)],
                        (1, 1): [qk_tile(0, 3, KSRC)],
                        (1, 2): [q1[0]], (1, 3): [q1[1]], (1, 4): [q1[2]],
                        (1, 5): [q1[3]], (1, 6): [q1[4]], (1, 7): [q1[5]],
                    })
                    attn_head(1, sched={(1, 0): [q1[6]], (1, 1): [q1[7]]})
                    attn_head(2)
                    attn_head(3)

    nc.finalize()
    return nc


_NC_CACHE = {}


def _get_nc():
    if "nc" not in _NC_CACHE:
        _NC_CACHE["nc"] = build_nc()
    return _NC_CACHE["nc"]


def make_in_maps(query, key, value, Wq, bq, Wk, bk, Wv, bv):
    query, key, value = (np.asarray(x, np.float32) for x in (query, key, value))
    Wq, Wk, Wv = (np.asarray(x, np.float32) for x in (Wq, Wk, Wv))
    bq, bk, bv = (np.asarray(x, np.float32) for x in (bq, bk, bv))
    mask = np.triu(np.ones((128, 128), np.float32)).astype(BFNP)

    def pack_small(bqs, bks, bvs, m):
        out = np.empty((128, 324), np.float32)
        out[:, 0:2] = bqs.reshape(2, 128).T
        out[:, 2:4] = bks.reshape(2, 128).T
        out[:, 4:260] = np.tile(bvs[None, :], (128, 1))
        out[:, 260:324] = np.ascontiguousarray(m).view(np.float32)
        return out

    in_maps = []
    for c in range(N_CORES):
        b, g = c // 2, c % 2
        sl = slice(DSL * g, DSL * g + DSL)
        in_maps.append(
            {
                "qT": np.ascontiguousarray(query[b].astype(BFNP).T),
                "kTd": np.ascontiguousarray(key[b].astype(BFNP).T),
                "vT": np.ascontiguousarray(value[b].astype(BFNP).T),
                "wqT": np.ascontiguousarray(Wq[sl].astype(BFNP).T),
                "wkT": np.ascontiguousarray(Wk[sl].astype(BFNP).T),
                "wvT": np.ascontiguousarray(Wv[sl].astype(BFNP).T),
                "smallp": pack_small(bq[sl], bk[sl], bv[sl], mask),
            }
        )
    return in_maps


def assemble_output(results):
    out = np.empty((B, S, D), np.float32)
    for c in range(N_CORES):
        b, g = c // 2, c % 2
        ot = results[c]["out_t"]  # [260, 2048]
        for hl in range(HPC):
            blk = ot[(HD + 1) * hl : (HD + 1) * hl + HD]  # [64, S]
            den = ot[(HD + 1) * hl + HD]  # [S]
            h = HPC * g + hl
            out[b, :, HD * h : HD * h + HD] = (blk / den).T
    return out


def run(trace=False, **inputs):
    nc = _get_nc()
    in_maps = make_in_maps(**inputs)
    res = run_bass_kernel_spmd(nc, in_maps, list(range(N_CORES)), trace=trace)
    return assemble_output(res.results), res


def kernel(**inputs) -> np.ndarray:
    out, _ = run(trace=False, **inputs)
    return out
